# revision 26
# baseline (speedup 1.0000x reference)
"""Ragged-sequence multi-head attention (B=16, S=1024, D=512, H=8, DH=64)
for 8 Trainium2 NeuronCores.

Strategy: data-parallel SPMD over a windowed ragged structure. The host
splits the 16 sequences into vseqs (contiguous q-tile ranges) and packs
them onto 8 cores; the shared program is parametrized by K-WINDOWS
(static k-tile ranges holding one parent sequence's K/V per core) and
Q-CHUNKS (static q-tile ranges bound to one window with a static k-cap).
Each core's in_map places its own parents' tokens into the static
layout, so all cores run one instruction stream over different data.

Masking: invalid key rows have zeroed x (V rows = 0, scores = 0 ->
exp = 1) and zeroed "validity" columns in the fused V|valid layout, so
both the numerator and the denominator are exact without any exp bias.

Mixed precision: Q/K projections fp16 -> QT/KT stored fp8e4 ->
DoubleRow fp8 scores (2x); exp on ACT (fp16 out); PV+denominator fused
as one fp16 [128,128] matmul per head (V dims | validity columns);
fp16 V/out projections.
"""

import math
import os

import numpy as np
import ml_dtypes

B, S, D = 16, 1024, 512
H, DH = 8, 64
N_CORES = 8
P = 128
KC = D // P  # 4

FP8 = ml_dtypes.float8_e4m3fn

_BUILD_CACHE: dict = {}


# --------------------------------------------------------------------------
# structure solver
# --------------------------------------------------------------------------

def _solve_structure(seq_lens):
    """Returns (windows, chunks, assign)."""
    import random

    nk = [max(1, math.ceil(int(l) / P)) for l in seq_lens]
    total = sum(k * k for k in nk)

    def mk_vseqs(T):
        vseqs = []
        for i, k in enumerate(nk):
            parts = max(1, math.ceil(k * k / T))
            base, rem, qt = k // parts, k - (k // parts) * parts, 0
            for p_ in range(parts):
                n = base + (1 if p_ < rem else 0)
                if n:
                    vseqs.append((i, qt, n, k))
                    qt += n
        return vseqs

    def cost_of(bins):
        M = max(len(b) for b in bins)
        Csum = NKT = NQ = 0
        for m in range(M):
            col = [sorted(b, key=lambda v: -v[3])[m] if len(b) > m else None
                   for b in bins]
            NKT += max((v[3] if v else 0) for v in col)
            counts = [v[2] if v else 0 for v in col]
            nks = [v[3] if v else 0 for v in col]
            caps = [max((nks[c] if i < counts[c] else 0) for c in range(8))
                    for i in range(max(counts))]
            Csum += sum(caps)
            NQ += len(caps)
        return 1536 * Csum + 3072 * NKT + 2560 * NQ

    best_bins, best_obj = None, None
    for t_mult in (0.85, 1.0, 1.2):
        T = max(4, math.ceil(total / 8 * t_mult))
        vs = sorted(mk_vseqs(T), key=lambda v: (-v[3], -v[2]))
        bins = [[] for _ in range(8)]
        loads = [0] * 8
        for v in vs:
            b = min(range(8), key=lambda j: loads[j])
            bins[b].append(v)
            loads[b] += v[2] * v[3]
        rng = random.Random(0)
        cur = cost_of(bins)
        for _ in range(3000):
            b1, b2 = rng.randrange(8), rng.randrange(8)
            if b1 == b2 or not bins[b1]:
                continue
            i1 = rng.randrange(len(bins[b1]))
            v1 = bins[b1][i1]
            if rng.random() < 0.5 and bins[b2]:
                i2 = rng.randrange(len(bins[b2]))
                v2 = bins[b2][i2]
                bins[b1][i1], bins[b2][i2] = v2, v1
                o = cost_of(bins)
                if o <= cur:
                    cur = o
                else:
                    bins[b1][i1], bins[b2][i2] = v1, v2
            else:
                bins[b1].pop(i1)
                bins[b2].append(v1)
                o = cost_of(bins)
                if o <= cur:
                    cur = o
                else:
                    bins[b2].pop()
                    bins[b1].insert(i1, v1)
        if best_obj is None or cur < best_obj:
            best_obj, best_bins = cur, [list(b) for b in bins]

    bins = [sorted(b, key=lambda v: -v[3]) for b in best_bins]
    M = max(len(b) for b in bins)
    windows = []
    chunks = []
    assign = [[None] * M for _ in range(8)]
    qoff = 0
    for m in range(M):
        col = [b[m] if len(b) > m else None for b in bins]
        windows.append(max((v[3] if v else 0) for v in col))
        for c in range(8):
            if col[c] is not None:
                assign[c][m] = (col[c][0], col[c][1], col[c][2])
        counts = [v[2] if v else 0 for v in col]
        nks = [v[3] if v else 0 for v in col]
        caps = [max((nks[c] if i < counts[c] else 0) for c in range(8))
                for i in range(max(counts))]
        # group into up-to-4-tile chunks with cap = group max (tail tiles
        # with smaller caps ride along: wider matmul streams amortize the
        # PE weight-load cost, and the masking keeps extra k-tiles exact)
        i = 0
        while i < len(caps):
            jx = min(len(caps), i + 4)
            chunks.append((qoff + i, jx - i, m, caps[i]))
            i = jx
        qoff += len(caps)
    return tuple(windows), tuple(chunks), assign


# --------------------------------------------------------------------------
# bass program
# --------------------------------------------------------------------------

def _build_bass(windows, chunks, debug=False):
    from contextlib import ExitStack

    import concourse.bass as bass
    import concourse.mybir as mybir
    import concourse.tile as tile
    from concourse import bacc

    fp32 = mybir.dt.float32
    fp16 = mybir.dt.float16
    fp8 = mybir.dt.float8e4
    Exp = mybir.ActivationFunctionType.Exp
    DR = mybir.MatmulPerfMode.DoubleRow
    mult = mybir.AluOpType.mult
    add = mybir.AluOpType.add

    NKT = sum(windows)
    NQ = sum(c[1] for c in chunks)
    NTOK_K = NKT * P
    NTOK_Q = NQ * P
    woff = [0]
    for w_ in windows:
        woff.append(woff[-1] + w_)

    nc = bacc.Bacc("TRN2", target_bir_lowering=False, debug=False)

    xq16_d = nc.dram_tensor("xq16", [P, KC, NTOK_Q], fp16, kind="ExternalInput").ap()
    xk16_d = nc.dram_tensor("xk16", [P, KC, NTOK_K], fp16, kind="ExternalInput").ap()
    wq16_d = nc.dram_tensor("wq16", [P, 2, 2, KC, P], fp16, kind="ExternalInput").ap()
    wk16_d = nc.dram_tensor("wk16", [P, 2, 2, KC, P], fp16, kind="ExternalInput").ap()
    wv16_d = nc.dram_tensor("wv16", [P, KC, D], fp16, kind="ExternalInput").ap()
    wo16_d = nc.dram_tensor("wo16", [P, KC, D], fp16, kind="ExternalInput").ap()
    vones_d = nc.dram_tensor("vones", [P, NKT, DH], fp16, kind="ExternalInput").ap()
    bo_d = nc.dram_tensor("bo", [D], fp32, kind="ExternalInput").ap()
    out_d = nc.dram_tensor("out", [NTOK_Q, D], fp16, kind="ExternalOutput").ap()
    if debug:
        dbg_qt = nc.dram_tensor("dbg_qt", [P, 2, 2, NTOK_Q], fp8, kind="ExternalOutput").ap()
        dbg_kt = nc.dram_tensor("dbg_kt", [P, 2, 2, NTOK_K], fp8, kind="ExternalOutput").ap()
        dbg_v = nc.dram_tensor("dbg_v", [P, NKT, H, P], fp16, kind="ExternalOutput").ap()
        dbg_ot = nc.dram_tensor("dbg_ot", [P, KC, NTOK_Q], fp16, kind="ExternalOutput").ap()

    with ExitStack() as ctx:
        tc = ctx.enter_context(tile.TileContext(nc))
        singles = ctx.enter_context(tc.tile_pool(name="singles", bufs=1))
        fpool = ctx.enter_context(tc.tile_pool(name="fpool", bufs=4))
        epool = ctx.enter_context(tc.tile_pool(name="epool", bufs=4))
        rpool = ctx.enter_context(tc.tile_pool(name="rpool", bufs=2))
        mmps = ctx.enter_context(tc.tile_pool(name="mmps", bufs=2, space="PSUM"))
        scps = ctx.enter_context(tc.tile_pool(name="scps", bufs=2, space="PSUM"))
        accps = ctx.enter_context(tc.tile_pool(name="accps", bufs=1, space="PSUM"))

        # ---- static inputs ----
        wq16 = singles.tile([P, 2, 2, KC, P], fp16)
        nc.sync.dma_start(out=wq16, in_=wq16_d)
        wk16 = singles.tile([P, 2, 2, KC, P], fp16)
        nc.sync.dma_start(out=wk16, in_=wk16_d)
        wv16 = singles.tile([P, KC, D], fp16)
        nc.sync.dma_start(out=wv16, in_=wv16_d)
        wo16 = singles.tile([P, KC, D], fp16)
        nc.sync.dma_start(out=wo16, in_=wo16_d)
        # per-window / per-chunk input tiles (one DMA each -> clean deps)
        xk16w = {}
        xq16c = {}
        bo_rep = singles.tile([P, D], fp32)
        bo_bcast = bass.AP(tensor=bo_d.tensor, offset=bo_d.offset,
                           ap=[[0, P], [1, D]])
        nc.gpsimd.dma_start(out=bo_rep, in_=bo_bcast)

        KT8 = singles.tile([P, 2, 2, NTOK_K], fp8, name="KT8")
        QT8 = singles.tile([P, 2, 2, NTOK_Q], fp8, name="QT8")
        V128 = singles.tile([P, NKT, H, P], fp16, name="V128")
        outT = singles.tile([P, KC, NTOK_Q], fp16, name="outT")

        # validity columns of V at 0:DH (denominator lands in PSUM rows
        # 0:64 because the custom-DVE reciprocal drops input partition
        # offsets); V dims at DH:P; DMA'd per window in first-use order

        # ---- projection unit emitters ----
        def qk_proj(dst, w16, x16, qs, w):
            for jj in range(2):
                for ii in range(2):
                    ps = mmps.tile([P, 512], fp32, name="qk_ps", tag="mm")
                    for kc in range(KC):
                        nc.tensor.matmul(
                            ps[:, :w],
                            w16[:, jj, ii, kc, :],
                            x16[:, kc, qs : qs + w],
                            start=(kc == 0), stop=(kc == KC - 1),
                        )
                    nc.vector.tensor_copy(
                        out=dst[:, jj, ii, qs : qs + w], in_=ps[:, :w]
                    )

        def v_proj(kt, m):
            x16 = xk16w[m]
            o = kt * P - woff[m] * P
            ps = mmps.tile([P, 512], fp32, name="v_ps", tag="mm")
            for kc in range(KC):
                nc.tensor.matmul(
                    ps,
                    x16[:, kc, o : o + P],
                    wv16[:, kc, :],
                    start=(kc == 0), stop=(kc == KC - 1),
                )
            nc.vector.tensor_copy(
                out=V128[:, kt, :, DH:P],
                in_=ps.rearrange("p (h d) -> p h d", h=H),
            )

        def o_proj(qt):
            ps = mmps.tile([P, 512], fp32, name="o_ps", tag="mm")
            for g in range(KC):
                nc.tensor.matmul(
                    ps,
                    outT[:, g, qt * P : (qt + 1) * P],
                    wo16[:, g, :],
                    start=(g == 0), stop=(g == KC - 1),
                )
            fout = fpool.tile([P, D], fp16, tag="fout")
            nc.vector.tensor_tensor(fout, ps, bo_rep, add)
            nc.sync.dma_start(out=out_d[qt * P : (qt + 1) * P, :], in_=fout)

        # ---- choreographed emission ----
        # PE must stay continuously busy (idle gaps drop it to the mid
        # p-state, halving throughput): attention batches software-pipeline
        # PVD one batch behind scores, and projection/output units drain
        # into the exp-wait gaps as fillers.
        fillers: list = []  # (tag, fn)
        emitted_tags = set()

        def fill(n):
            for _ in range(min(n, len(fillers))):
                tag, fn = fillers.pop(0)
                emitted_tags.add(tag)
                fn()

        def ensure_ready(tags):
            need = set(tags) - emitted_tags
            while need & {t for t, _ in fillers} or (
                need and any(t in need for t, _ in fillers)
            ):
                tag, fn = fillers.pop(0)
                emitted_tags.add(tag)
                fn()
                need -= {tag}

        # execution order: smallest window first (fast start), largest in
        # the middle, a small one last (short drain tail)
        worder = sorted(range(len(windows)), key=lambda m: windows[m])
        corder = [ch for m in worder for ch in chunks if ch[2] == m]

        # input DMAs split per window / chunk, emitted in first-use order
        nc.sync.dma_start(out=wk16, in_=wk16_d)
        m0 = worder[0]
        s0, s1 = woff[m0] * P, (woff[m0] + windows[m0]) * P
        t = singles.tile([P, KC, s1 - s0], fp16, name=f"xk16_{m0}")
        nc.sync.dma_start(out=t, in_=xk16_d[:, :, s0:s1])
        xk16w[m0] = t
        nc.sync.dma_start(out=wv16, in_=wv16_d)
        # ship one head's validity columns, replicate on-chip (1->2->4->8)
        nc.sync.dma_start(out=V128[:, :, 0, 0:DH], in_=vones_d)
        nc.vector.tensor_copy(out=V128[:, :, 1, 0:DH], in_=V128[:, :, 0, 0:DH])
        nc.vector.tensor_copy(out=V128[:, :, 2:4, 0:DH], in_=V128[:, :, 0:2, 0:DH])
        nc.vector.tensor_copy(out=V128[:, :, 4:8, 0:DH], in_=V128[:, :, 0:4, 0:DH])
        nc.sync.dma_start(out=wq16, in_=wq16_d)
        c0 = corder[0]
        s0, s1 = c0[0] * P, (c0[0] + c0[1]) * P
        t = singles.tile([P, KC, s1 - s0], fp16, name=f"xq16_{c0[0]}")
        nc.sync.dma_start(out=t, in_=xq16_d[:, :, s0:s1])
        xq16c[c0[0]] = t
        for m in worder[1:]:
            s0, s1 = woff[m] * P, (woff[m] + windows[m]) * P
            t = singles.tile([P, KC, s1 - s0], fp16, name=f"xk16_{m}")
            nc.sync.dma_start(out=t, in_=xk16_d[:, :, s0:s1])
            xk16w[m] = t
        for (qt_off, ntiles, m, cap) in corder[1:]:
            s0, s1 = qt_off * P, (qt_off + ntiles) * P
            t = singles.tile([P, KC, s1 - s0], fp16, name=f"xq16_{qt_off}")
            nc.sync.dma_start(out=t, in_=xq16_d[:, :, s0:s1])
            xq16c[qt_off] = t
        nc.sync.dma_start(out=wo16, in_=wo16_d)
        nc.gpsimd.dma_start(out=bo_rep, in_=bo_bcast)

        def qk_unit(dst, w16, x16, xoff, qs, w, jj, ii):
            ps = mmps.tile([P, 512], fp32, name="qk_ps", tag="mm")
            for kc in range(KC):
                nc.tensor.matmul(
                    ps[:, :w],
                    w16[:, jj, ii, kc, :],
                    x16[:, kc, qs - xoff : qs - xoff + w],
                    start=(kc == 0), stop=(kc == KC - 1),
                )
            nc.vector.tensor_copy(
                out=dst[:, jj, ii, qs : qs + w], in_=ps[:, :w]
            )

        kq_done = set()

        def emit_now(tag_units):
            tag, units = tag_units
            kq_done.add(tag)
            for u in units:
                u()

        def k_units(m):
            us = []
            for qs in range(woff[m] * P, (woff[m] + windows[m]) * P, 512):
                w_ = min(512, (woff[m] + windows[m]) * P - qs)
                def one(qs=qs, w_=w_, m=m):
                    for jj in range(2):
                        for ii in range(2):
                            qk_unit(KT8, wk16, xk16w[m], woff[m] * P,
                                    qs, w_, jj, ii)
                us.append(one)
            return us

        def v_units(m):
            return [lambda kt=kt, m=m: v_proj(kt, m)
                    for kt in range(woff[m], woff[m] + windows[m])]

        def q_units(qt_off, ntiles):
            def one(qt_off=qt_off, ntiles=ntiles):
                for jj in range(2):
                    for ii in range(2):
                        qk_unit(QT8, wq16, xq16c[qt_off], qt_off * P,
                                qt_off * P, ntiles * P, jj, ii)
            return [one]

        first = corder[0]
        emit_now((("k", first[2]), k_units(first[2])))
        emit_now((("v", first[2]), v_units(first[2])))
        emit_now((("q", first[0]), q_units(first[0], first[1])))
        for ch in corder[1:]:
            m = ch[2]
            if ("k", m) not in kq_done:
                kq_done.add(("k", m))
                fillers.extend((("k", m), u) for u in k_units(m))
            if ("v", m) not in kq_done:
                kq_done.add(("v", m))
                fillers.extend((("v", m), u) for u in v_units(m))
            if ("q", ch[0]) not in kq_done:
                kq_done.add(("q", ch[0]))
                fillers.extend((("q", ch[0]), u)
                               for u in q_units(ch[0], ch[1]))

        batch_count = [0]
        for (qt_off_, ntiles_, m_, cap_) in chunks:
            ktc_ = max(1, 512 // (ntiles_ * P))
            batch_count[0] += 4 * ((cap_ + ktc_ - 1) // ktc_)
        batches_left = [batch_count[0]]

        pending_norm: list = []

        def emit_norm():
            while pending_norm:
                norm_fn, post = pending_norm.pop(0)
                norm_fn()
                fillers.extend((("o", id(p)), p) for p in post)

        def attn_chunk(qt_off, ntiles, m, cap):
            qs, w = qt_off * P, ntiles * P
            ktc = max(1, 512 // w)  # k-tiles per scores tile
            bs = 512 if w == 384 else w  # PSUM-bank-aligned block stride
            for hl in range(4):
                o_pd = accps.tile([P, 2, 512], fp32, name="o_pd", tag="o_pd")

                def scores_exp(kt0, kn):
                    s_t = scps.tile([P, 1024], fp32, name="s_t", tag="s_t")
                    e_t = epool.tile([P, 1024], fp16, name="e_t", tag="e_t")
                    for dk in range(kn):
                        ktg = woff[m] + kt0 + dk
                        for jj in range(2):
                            off = (dk * 2 + jj) * bs
                            nc.tensor.matmul(
                                s_t[:, off : off + w],
                                KT8[32 * hl : 32 * hl + 32, jj, :,
                                    ktg * P : (ktg + 1) * P],
                                QT8[32 * hl : 32 * hl + 32, jj, :, qs : qs + w],
                                start=True, stop=True, perf_mode=DR,
                                tile_position=(32 * hl, 0),
                            )
                    if bs == w:
                        nc.scalar.activation(
                            e_t[:, 0 : kn * 2 * w], s_t[:, 0 : kn * 2 * w],
                            Exp, scale=0.125,
                        )
                    else:
                        nc.scalar.activation(
                            e_t.rearrange("p (j q) -> p j q", j=2)[:, :, :w],
                            s_t.rearrange("p (j q) -> p j q", j=2)[:, :, :w],
                            Exp, scale=0.125,
                        )
                    return e_t

                def pvd(kt0, kn, e_t):
                    for dk in range(kn):
                        ktg = woff[m] + kt0 + dk
                        kt = kt0 + dk
                        for jj in range(2):
                            off = (dk * 2 + jj) * bs
                            nc.tensor.matmul(
                                o_pd[:, jj, :w],
                                V128[:, ktg, hl + 4 * jj, :],
                                e_t[:, off : off + w],
                                start=(kt == 0), stop=(kt == cap - 1),
                                skip_group_check=True,
                            )

                pend = None
                for kt0 in range(0, cap, ktc):
                    kn = min(ktc, cap - kt0)
                    e_t = scores_exp(kt0, kn)
                    if pend is None:
                        # batch 0 of this hl: place the previous normalize
                        # (DVE) into the exp-wait gap
                        emit_norm()
                    else:
                        pvd(*pend)
                    nfill = -(-len(fillers) // max(1, batches_left[0]))
                    fill(nfill)
                    batches_left[0] -= 1
                    pend = (kt0, kn, e_t)
                pvd(*pend)

                def norm(hl=hl, o_pd=o_pd):
                    rrep = rpool.tile([DH, 2, 512], fp32, tag="rrep")
                    nc.vector.reciprocal_approx_fast(
                        out=rrep[:, :, :w], in_=o_pd[0:DH, :, :w]
                    )
                    for jj in range(2):
                        nc.vector.tensor_tensor(
                            outT[DH * jj : DH * jj + DH, hl, qs : qs + w],
                            o_pd[DH:P, jj, :w],
                            rrep[:, jj, :w],
                            mult,
                        )

                post = []
                if hl == 3:
                    post = [
                        (lambda qt=qt: o_proj(qt))
                        for qt in range(qt_off, qt_off + ntiles)
                    ]
                pending_norm.append((norm, post))

        for ci, (qt_off, ntiles, m, cap) in enumerate(corder):
            ensure_ready([("k", m), ("v", m), ("q", qt_off)])
            attn_chunk(qt_off, ntiles, m, cap)
        emit_norm()
        while fillers:
            tag, fn = fillers.pop(0)
            fn()

        if debug:
            nc.sync.dma_start(out=dbg_qt, in_=QT8)
            nc.sync.dma_start(out=dbg_kt, in_=KT8)
            nc.sync.dma_start(out=dbg_v, in_=V128)
            nc.sync.dma_start(out=dbg_ot, in_=outT)

    nc.compile()
    return nc


def _get_program(windows, chunks, debug):
    key = (windows, chunks, debug)
    if key not in _BUILD_CACHE:
        _BUILD_CACHE[key] = _build_bass(windows, chunks, debug)
    return _BUILD_CACHE[key]


# --------------------------------------------------------------------------
# host glue
# --------------------------------------------------------------------------

def _xt(tokens_x):
    """[T, D] fp32 -> [P, KC, T] transposed layout."""
    t = tokens_x.T.reshape(KC, P, tokens_x.shape[0]).transpose(1, 0, 2)
    return np.ascontiguousarray(t)


def _first_qt(chunks, m):
    for (qt_off, ntiles, mm, cap) in chunks:
        if mm == m:
            return qt_off
    raise ValueError(m)


def kernel(x, seq_lens, Wq, Wk, Wv, Wo, bo) -> np.ndarray:
    from concourse.bass_utils import run_bass_kernel_spmd

    x = np.asarray(x, dtype=np.float32)
    seq_lens_np = np.asarray(seq_lens, dtype=np.int32)
    Wq = np.asarray(Wq, dtype=np.float32)
    Wk = np.asarray(Wk, dtype=np.float32)
    Wv = np.asarray(Wv, dtype=np.float32)
    Wo = np.asarray(Wo, dtype=np.float32)
    bo = np.asarray(bo, dtype=np.float32)

    windows, chunks, assign = _solve_structure(seq_lens_np)
    NKT = sum(windows)
    NQ = sum(c[1] for c in chunks)
    woff = [0]
    for w_ in windows:
        woff.append(woff[-1] + w_)

    debug = bool(int(os.environ.get("KERNEL_DEBUG", "0")))
    nc = _get_program(windows, chunks, debug)

    # weight pre-arrangement (shared across cores)
    pidx = np.arange(P)
    hl_of = pidx // 32
    dlow = pidx % 32
    col = np.zeros((2, 2, P), dtype=np.int64)
    for jj in range(2):
        for ii in range(2):
            col[jj, ii] = 64 * (hl_of + 4 * jj) + dlow + 32 * ii

    def arrange_qk(W):
        # [c(128), j, i, kc, m] = W[kc*128+c, col(m,j,i)]
        a = np.zeros((P, 2, 2, KC, P), dtype=np.float32)
        for kc in range(KC):
            rows = np.arange(P) + kc * 128
            for jj in range(2):
                for ii in range(2):
                    a[:, jj, ii, kc, :] = W[rows[:, None], col[jj, ii][None, :]]
        return a.astype(np.float16)

    wq16 = arrange_qk(Wq)
    wk16 = arrange_qk(Wk)
    wv16 = np.ascontiguousarray(
        Wv.reshape(KC, P, D).transpose(1, 0, 2)
    ).astype(np.float16)
    wo16 = np.zeros((P, KC, D), dtype=np.float32)
    for g in range(KC):
        rows = 64 * (g + 4 * (pidx // 64)) + pidx % 64
        wo16[:, g, :] = Wo[rows, :]
    wo16 = wo16.astype(np.float16)

    in_maps = []
    for c in range(N_CORES):
        xk = np.zeros((NKT * P, D), dtype=np.float32)
        vones = np.zeros((P, NKT, DH), dtype=np.float16)
        for m, a in enumerate(assign[c]):
            if a is None:
                continue
            seq = a[0]
            L = int(seq_lens_np[seq])
            nkt_par = min(math.ceil(L / P), windows[m])
            n = min(L, nkt_par * P)
            xk[woff[m] * P : woff[m] * P + n] = x[seq, :n]
            pos = (np.arange(windows[m])[None, :] * P
                   + np.arange(P)[:, None])
            valid = (pos < L).astype(np.float16)  # [P, win]
            vones[:, woff[m] : woff[m] + windows[m], :] = valid[:, :, None]
        xq = np.zeros((NQ * P, D), dtype=np.float32)
        for (qt_off, ntiles, m, cap) in chunks:
            a = assign[c][m]
            if a is None:
                continue
            seq, qt0, njobs = a
            L = int(seq_lens_np[seq])
            for idx in range(ntiles):
                gidx = qt_off + idx
                job = gidx - _first_qt(chunks, m)
                if job < njobs:
                    r0 = (qt0 + job) * P
                    n = max(0, min(L - r0, P))
                    if n > 0:
                        xq[gidx * P : gidx * P + n] = x[seq, r0 : r0 + n]
        in_maps.append({
            "xq16": _xt(xq).astype(np.float16),
            "xk16": _xt(xk).astype(np.float16),
            "wq16": wq16, "wk16": wk16, "wv16": wv16, "wo16": wo16,
            "vones": vones, "bo": bo,
        })

    trace = bool(int(os.environ.get("KERNEL_TRACE", "0")))
    res = run_bass_kernel_spmd(
        nc, in_maps, core_ids=list(range(N_CORES)), trace=trace
    )
    kernel.last_results = res

    out = np.zeros((B, S, D), dtype=np.float32)
    for c in range(N_CORES):
        o = res.results[c]["out"].astype(np.float32)
        for (qt_off, ntiles, m, cap) in chunks:
            a = assign[c][m]
            if a is None:
                continue
            seq, qt0, njobs = a
            L = int(seq_lens_np[seq])
            for idx in range(ntiles):
                gidx = qt_off + idx
                job = gidx - _first_qt(chunks, m)
                if job < njobs:
                    r0 = (qt0 + job) * P
                    n = max(0, min(L - r0, P))
                    if n > 0:
                        out[seq, r0 : r0 + n] = o[gidx * P : gidx * P + n]
    return out
